# revision 28
# baseline (speedup 1.0000x reference)
"""Trainium2 Bass kernel for nn_PatternBranch (conv3x3/s2+relu -> routed heads).

Strategy
--------
Everything after the conv folds into ONE matmul: with feats0 flattened as
(position p, channel co), the base head, the pattern head (channel-gather
scatter-added over pattern_set_index), and the match head (GAP dot match_w
== sum over (p, co) of feats0 * match_w[co]/1024) concatenate into a single
fused weight W5[p, co, 0:5].  The device then computes, per core:

    conv (im2col K=27 matmul)  ->  PSUM  -> relu-evict (ACT+DVE split)
    -> feats0 in SBUF          ->  fused matmul  ->  partial logits [5, 256]

Sharding: the 1024 output positions are split 8 ways (128 positions /
core = 4 rows of the 32x32 output grid); every core sees all 256 samples.
Each fused matmul packs TWO positions (M=10, N=512 sample-columns; the
block-cross terms land in ignored PSUM quadrants), and W5 shards cleanly.
Host sums the 8 partial logits and runs the tiny [256,5] epilogue
(sigmoid/softmax/route).

Matmul inputs are bf16 (PE streams 1 col/cycle at 2.4 GHz warm; fp32/fp32r
paths cap at ~1.2 GHz effective).  PSUM accumulation is fp32.  Conv matmuls
are row-tiled (K=27 strips at partitions 32q) so up to 4 run concurrently;
dummy warm-up matmuls open the HAM clock gate before the real work; the
ReLU eviction is split across ScalarE and VectorE and the fused matmuls
trail their evictions by two steps so the PE never stalls.
"""
import sys

for _p in ("/opt/trn_rl_repo", "/root/.axon_site/_ro/trn_rl_repo"):
    if _p not in sys.path:
        sys.path.append(_p)

import numpy as np
import ml_dtypes

import concourse.bacc as bacc
import concourse.mybir as mybir
import concourse.tile as tile
from concourse.bass_utils import run_bass_kernel_spmd

F32 = mybir.dt.float32
BF16 = mybir.dt.bfloat16
NP_BF16 = ml_dtypes.bfloat16

B = 256          # batch
HW_IN = 64       # input spatial
CIN = 3
COUT = 128
KPAT = 32        # gathered channels for pattern head
P_GRID = 32      # output spatial (stride 2)
N_CORES = 8
P_CORE = 128     # positions per core (4 rows of 32)
QB = 64          # samples per quarter
NQ = 4           # quarters (4 x 64 = 256 samples)
KC = 27          # im2col contraction (3*3*3)
NT = 16          # time steps: 8 positions x 4 quarters each

_NC_CACHE = {}


def _build_nc():
    """One SPMD program, same for all 8 cores."""
    nc = bacc.Bacc("TRN2", target_bir_lowering=False, debug=False)

    imcol = nc.dram_tensor("imcol", [NQ, KC, P_CORE * QB], BF16,
                           kind="ExternalInput")
    cw = nc.dram_tensor("cw", [KC, COUT], BF16, kind="ExternalInput")
    convb = nc.dram_tensor("convb", [COUT, 1], F32, kind="ExternalInput")
    w5 = nc.dram_tensor("w5", [COUT, P_CORE * 5], BF16, kind="ExternalInput")
    outp = nc.dram_tensor("out", [10, 2 * B], F32, kind="ExternalOutput")

    with tile.TileContext(nc) as tc:
        with tc.tile_pool(name="singles", bufs=1) as singles, \
             tc.tile_pool(name="convps", bufs=3, space="PSUM") as convps, \
             tc.tile_pool(name="faccps", bufs=1, space="PSUM") as faccps:

            # DMAs on sync + gpsimd only, earliest-needed first; the ACT
            # queue stays DMA-free so its table-load + evicts start early.
            cw_sb = singles.tile([128, COUT], BF16)
            convb_sb = singles.tile([COUT, 1], F32)
            imq = singles.tile([128, P_CORE * QB], BF16)
            w5_sb = singles.tile([COUT, P_CORE * 5], BF16)
            CH = P_CORE * QB // 2

            # preload the ACT function table with a cheap activation so the
            # first real evict doesn't pay the ~2.7us table load.
            actwarm = singles.tile([128, 1], F32)
            nc.vector.memset(actwarm[:, :], 0.0)
            nc.scalar.activation(
                out=actwarm[:, :], in_=actwarm[:, :],
                func=mybir.ActivationFunctionType.Relu, bias=0.0, scale=1.0)

            def imq_dma(eng, q, lo, hi):
                eng.dma_start(
                    out=imq[32 * q:32 * q + KC, lo:hi],
                    in_=imcol[q, :, lo:hi])

            # first conv pair needs imq q0/q1 head + cw strips 0/32
            J0 = 1024
            imq_dma(nc.sync, 0, 0, J0)
            imq_dma(nc.gpsimd, 1, 0, J0)
            nc.sync.dma_start(out=cw_sb[0:KC, :], in_=cw[:, :])
            nc.sync.dma_start(out=cw_sb[32:32 + KC, :], in_=cw[:, :])
            nc.gpsimd.dma_start(out=cw_sb[64:64 + KC, :], in_=cw[:, :])
            nc.gpsimd.dma_start(out=cw_sb[96:96 + KC, :], in_=cw[:, :])
            imq_dma(nc.sync, 2, 0, J0)
            imq_dma(nc.gpsimd, 3, 0, J0)
            nc.scalar.dma_start(out=convb_sb[:, :], in_=convb[:, :])
            nc.scalar.dma_start(out=w5_sb[:, :], in_=w5[:, :])
            imq_dma(nc.sync, 0, J0, CH)
            imq_dma(nc.gpsimd, 1, J0, CH)
            imq_dma(nc.sync, 2, J0, CH)
            imq_dma(nc.gpsimd, 3, J0, CH)
            imq_dma(nc.sync, 0, CH, 2 * CH)
            imq_dma(nc.gpsimd, 1, CH, 2 * CH)
            imq_dma(nc.sync, 2, CH, 2 * CH)
            imq_dma(nc.gpsimd, 3, CH, 2 * CH)

            # PE warmup: dummy matmuls on a zeroed tile so the HAM clock
            # gate opens (~3.4us sustained busy) before the real matmuls.
            zdummy = singles.tile([128, 512], BF16)
            nc.vector.memset(zdummy[:, :], 0.0)
            warm_ps = faccps.tile([128, 512], F32, tag="warm")
            for _ in range(16):
                nc.tensor.matmul(warm_ps[:, :], zdummy[:, 0:128],
                                 zdummy[:, :], start=True, stop=True)

            # feats0[co, q, p, b]  (position-major within quarter)
            feats0 = singles.tile([COUT, NQ, P_CORE, QB], BF16)

            facc0 = faccps.tile([128, 2 * B], F32)
            out_sb = singles.tile([128, 2 * B], F32)

            # ACT/DVE split for the 32 relu-evictions (measured rates)
            t_act, t_dve = 0.1, 0.0
            evict_engine = []
            for _ in range(2 * NT):
                if t_act + 1112.0 <= t_dve + 1281.0:
                    evict_engine.append("act"); t_act += 1112.0
                else:
                    evict_engine.append("dve"); t_dve += 1281.0

            import concourse.bass as bass

            def fused_step(t):
                # fused matmul: 2 positions packed per MM (M=10, N=512).
                # Cross terms (W5[p] x feats0[p+1] etc.) accumulate into the
                # ignored quadrants of the [10, 512] accumulator.
                for dp in range(0, 8, 2):
                    p = 8 * t + dp
                    f = feats0[:, :, p, :]
                    rhs = bass.AP(
                        tensor=f.tensor, offset=f.offset,
                        ap=[f.ap[0], [QB, 2], f.ap[1], f.ap[2]])
                    nc.tensor.matmul(
                        facc0[0:10, :],
                        w5_sb[:, 5 * p:5 * p + 10],
                        rhs,
                        start=(p == 0), stop=(p == P_CORE - 2))

            for t in range(NT):
                for pair in range(2):
                    ps = convps.tile([128, 2, 512], F32, tag="convps")
                    for qi in range(2):
                        q = 2 * pair + qi
                        nc.tensor.matmul(
                            ps[:, qi, :],
                            cw_sb[32 * q:32 * q + KC, :],
                            imq[32 * q:32 * q + KC, 512 * t:512 * (t + 1)],
                            start=True, stop=True,
                            tile_position=(32 * q, 0))
                    # relu + bias eviction PSUM -> SBUF (bf16)
                    dst = feats0[:, 2 * pair:2 * pair + 2, 8 * t:8 * t + 8, :]
                    eng = evict_engine[2 * t + pair]
                    if eng == "act":
                        nc.scalar.activation(
                            out=dst, in_=ps[:, :, :],
                            func=mybir.ActivationFunctionType.Relu,
                            bias=convb_sb[:, 0:1], scale=1.0)
                    else:
                        nc.vector.tensor_scalar(
                            dst, ps[:, :, :],
                            convb_sb[:, 0:1], 0.0,
                            mybir.AluOpType.add, mybir.AluOpType.max)
                # fused matmuls run two steps behind their evicts so the PE
                # never stalls on eviction completion jitter.
                if t >= 2:
                    fused_step(t - 2)
            fused_step(NT - 2)
            fused_step(NT - 1)

            nc.vector.tensor_copy(out=out_sb[0:10, :], in_=facc0[0:10, :])
            nc.sync.dma_start(out=outp[:, :], in_=out_sb[0:10, :])

    nc.compile()
    return nc


def get_nc():
    if "nc" not in _NC_CACHE:
        _NC_CACHE["nc"] = _build_nc()
    return _NC_CACHE["nc"]


def _host_prep(inputs, conv_w, match_w, pat_w, base_w, pattern_set_index):
    """Build per-core im2col + fused weight arrays."""
    x = np.ascontiguousarray(np.asarray(inputs, dtype=np.float32))
    # SAME padding for k=3 s=2 on 64 -> pad (0, 1)
    xp = np.zeros((B, HW_IN + 1, HW_IN + 1, CIN), np.float32)
    xp[:, :HW_IN, :HW_IN, :] = x
    s = xp.strides
    win = np.lib.stride_tricks.as_strided(
        xp, shape=(B, P_GRID, P_GRID, 3, 3, CIN),
        strides=(s[0], 2 * s[1], 2 * s[2], s[1], s[2], s[3]))
    # [k, p_global, b]
    imcol = np.ascontiguousarray(win.transpose(3, 4, 5, 1, 2, 0)).reshape(
        KC, P_GRID * P_GRID, B)
    # [core, q, k, p_local, b_q] -> [8, 4, 27, 8192] bf16
    A = np.ascontiguousarray(
        imcol.reshape(KC, N_CORES, P_CORE, NQ, QB).transpose(1, 3, 0, 2, 4)
        .astype(NP_BF16)
    ).reshape(N_CORES, NQ, KC, P_CORE * QB)

    cwr = np.ascontiguousarray(
        np.asarray(conv_w, np.float32).reshape(KC, COUT).astype(NP_BF16))

    # fused weight: [p, co, 5] = [base(3) | pat scatter | match/1024]
    base_w3 = np.asarray(base_w, np.float32).reshape(P_GRID * P_GRID, COUT, 3)
    pat_w2 = np.asarray(pat_w, np.float32).reshape(P_GRID * P_GRID, KPAT)
    idx = np.asarray(pattern_set_index).astype(np.int64)
    pw_sc = np.zeros((P_GRID * P_GRID, COUT), np.float32)
    np.add.at(pw_sc,
              (np.repeat(np.arange(P_GRID * P_GRID), KPAT),
               np.tile(idx, P_GRID * P_GRID)),
              pat_w2.ravel())
    W5 = np.zeros((P_GRID * P_GRID, COUT, 5), np.float32)
    W5[:, :, 0:3] = base_w3
    W5[:, :, 3] = pw_sc
    W5[:, :, 4] = np.asarray(match_w, np.float32)[None, :] / float(P_GRID * P_GRID)
    # per-core: [co, p_local, 5] -> [128, 640] bf16
    W5c = np.ascontiguousarray(
        W5.reshape(N_CORES, P_CORE, COUT, 5).transpose(0, 2, 1, 3)
        .astype(NP_BF16)
    ).reshape(N_CORES, COUT, P_CORE * 5)

    return A, cwr, W5c


def kernel(inputs, conv_w, conv_b, match_w, match_b,
           pat_w, pat_b, base_w, base_b, pattern_set_index):
    A, cwr, W5c = _host_prep(inputs, conv_w, match_w, pat_w, base_w,
                             pattern_set_index)
    convb2 = np.ascontiguousarray(
        np.asarray(conv_b, np.float32).reshape(COUT, 1))

    nc = get_nc()
    in_maps = [
        {"imcol": A[c], "cw": cwr, "convb": convb2, "w5": W5c[c]}
        for c in range(N_CORES)
    ]
    res = run_bass_kernel_spmd(nc, in_maps, core_ids=list(range(N_CORES)))

    acc = np.zeros((5, B), np.float64)
    for c in range(N_CORES):
        o = res.results[c]["out"].astype(np.float64)  # [10, 2B] packed pairs
        acc += o[0:5, 0:B] + o[5:10, B:2 * B]
    logits = acc.T  # [B, 5]

    # epilogue (host, [256, 5] only)
    base_logits = logits[:, 0:3] + np.asarray(base_b, np.float64)[None, :]
    plogit = logits[:, 3] + float(np.asarray(pat_b).reshape(-1)[0])
    mlogit = logits[:, 4] + float(np.asarray(match_b).reshape(-1)[0])
    p = 1.0 / (1.0 + np.exp(-plogit))
    e = np.exp(base_logits - base_logits.max(axis=1, keepdims=True))
    base = e / e.sum(axis=1, keepdims=True)
    o = (1.0 - p) * 0.5
    cat = np.stack([p, o, o], axis=-1)
    use_pat = (mlogit > 0.0) & (p >= 0.5)
    out = np.where(use_pat[:, None], cat, base)
    return out.astype(np.float32)
